# revision 71
# baseline (speedup 1.0000x reference)
"""PVT-style spatial-reduction attention on 8 TRN2 NeuronCores.

Sharding: core c -> (batch b = c//2, head-group g = c%2), 4 heads each.
No collectives: each core computes a partial projection output
outT_partial (512, 4096); host sums the two partials per batch.

On-core orientation: features-on-partition, tokens-on-free throughout:
  xT (ch, tok) -> convT (oc, pos) -> LN -> kT (kc, pos) / v (pos, vc)
  qT (qc, tok); scoresT (kv, tok) = kT_h^T-slice @ qT_h; exp on ACT;
  avT (65, tok) via v4 lhsT with ones column (row 64 = softmax denom);
  netT = avT * bcast(1/denom); outT = proj_w_g^T-slice @ netT.
All matmuls bf16 (f32 PSUM) except tiny f32 broadcast matmuls.
"""
import sys as _sys
for _p in ("/opt/trn_rl_repo", "/opt/pypackages"):
    if _p not in _sys.path:
        _sys.path.insert(0, _p)

import numpy as np
import ml_dtypes
from contextlib import ExitStack

import concourse.bass as bass
import concourse.mybir as mybir
import concourse.tile as tile
from concourse import bacc
from concourse.bass_utils import run_bass_kernel_spmd

BF = mybir.dt.bfloat16
F32 = mybir.dt.float32
P = 128
BS, N, DIM, HEADS, HD = 4, 4096, 512, 8, 64
NKV = 1024
SCALE = HD ** -0.5  # 0.125
EPS = 1e-5
NCH = 8          # token chunks of 512
CHUNK = N // NCH  # 512
NBF = np.dtype(ml_dtypes.bfloat16)


DEBUG = False


def build_nc():
    nc = bacc.Bacc()
    xt_d = nc.declare_dram_parameter("xt", (4, P, N), BF, isOutput=False)
    wc_d = nc.declare_dram_parameter("wc", (16, P, DIM), BF, isOutput=False)
    qw_d = nc.declare_dram_parameter("qw", (4, P, 256), BF, isOutput=False)
    kw_d = nc.declare_dram_parameter("kw", (4, P, 256), BF, isOutput=False)
    vw_d = nc.declare_dram_parameter("vw", (4, P, 256), BF, isOutput=False)
    pw_d = nc.declare_dram_parameter("pw", (2, P, DIM), BF, isOutput=False)
    b4_d = nc.declare_dram_parameter("b4", (P, 4), F32, isOutput=False)
    kb_d = nc.declare_dram_parameter("kb", (P, 2), F32, isOutput=False)
    vb_d = nc.declare_dram_parameter("vb", (1, 256), BF, isOutput=False)
    out_d = nc.declare_dram_parameter("out", (DIM, N), F32, isOutput=True)
    if DEBUG:
        dbg_xrn = nc.declare_dram_parameter("dbg_xrn", (P, 4, NKV), F32, isOutput=True)
        dbg_kT2 = nc.declare_dram_parameter("dbg_kT2", (64, 4, NKV), F32, isOutput=True)
        dbg_qT2 = nc.declare_dram_parameter("dbg_qT2", (64, 4, N), F32, isOutput=True)
        dbg_v4 = nc.declare_dram_parameter("dbg_v4", (P, 8, 4, 65), F32, isOutput=True)
        dbg_net = nc.declare_dram_parameter("dbg_net", (P, 2, N), F32, isOutput=True)
        dbg_av = nc.declare_dram_parameter("dbg_av", (P, CHUNK), F32, isOutput=True)
        dbg_avu = nc.declare_dram_parameter("dbg_avu", (P, CHUNK), F32, isOutput=True)
        dbg_rb = nc.declare_dram_parameter("dbg_rb", (P, CHUNK), F32, isOutput=True)
        dbg_e = nc.declare_dram_parameter("dbg_e", (P, 4, CHUNK), F32, isOutput=True)

    with tile.TileContext(nc) as tc, ExitStack() as ctx:
        persist = ctx.enter_context(tc.tile_pool(name="persist", bufs=1))

        # ---- persistent SBUF tensors
        xt = persist.tile([P, 4, N], BF, tag="xt")
        wc = persist.tile([P, 16, DIM], BF, tag="wc")
        qw = persist.tile([P, 4, 256], BF, tag="qw")
        kw = persist.tile([P, 4, 256], BF, tag="kw")
        vw = persist.tile([P, 4, 256], BF, tag="vw")
        pw = persist.tile([P, 2, DIM], BF, tag="pw")
        b4 = persist.tile([P, 4], F32, tag="b4")
        kb = persist.tile([P, 2], F32, tag="kb")
        vb = persist.tile([1, 256], BF, tag="vb")

        xrc = persist.tile([P, 4, NKV], BF, tag="xrc")    # centered conv out
        xrn = persist.tile([P, 4, NKV], BF, tag="xrn")    # LN'd
        qT2 = persist.tile([64, 4, N], BF, tag="qT2")     # per-head q rows
        kT2 = persist.tile([64, 4, NKV], BF, tag="kT2")   # per-head k rows
        v4 = persist.tile([P, 8, 4, 65], BF, tag="v4")    # v + ones col
        netT = persist.tile([P, 2, N], BF, tag="netT")
        rstd = persist.tile([1, NKV], F32, tag="rstd")
        stdt = persist.tile([1, NKV], F32, tag="stdt")

        ones_inv = persist.tile([P, 1], BF, tag="ones_inv")   # 1/512 column (K=128, M=1)
        ones128f = persist.tile([1, P], F32, tag="ones128f")  # f32 ones row (K=1, M=128)
        ones128b = persist.tile([1, P], BF, tag="ones128b")   # bf16 ones row (K=1, M=128)
        mask0 = persist.tile([1, P], BF, tag="mask0")
        mask1 = persist.tile([1, P], BF, tag="mask1")
        eps1 = persist.tile([1, 1], F32, tag="eps1")

        # ---- DMAs: qw first (q fills conv's DMA stalls), xt in quarters,
        # conv weights, then later-stage weights
        nc.sync.dma_start(b4[:], b4_d[:])
        nc.sync.dma_start(kb[:], kb_d[:])
        nc.sync.dma_start(vb[:], vb_d[:])
        for kt in range(4):
            nc.sync.dma_start(qw[:, kt, :], qw_d[kt])
            nc.sync.dma_start(xt[:, kt, 0:1024], xt_d[kt][:, 0:1024])
        for kt in range(4):
            nc.sync.dma_start(xt[:, kt, 1024:2048], xt_d[kt][:, 1024:2048])
        for kt in range(4):
            for d in range(4):
                nc.sync.dma_start(wc[:, d * 4 + kt, :], wc_d[d * 4 + kt])
        for q4 in range(2, 4):
            for kt in range(4):
                nc.sync.dma_start(xt[:, kt, q4 * 1024:(q4 + 1) * 1024],
                                  xt_d[kt][:, q4 * 1024:(q4 + 1) * 1024])
        for kt in range(4):
            nc.sync.dma_start(kw[:, kt, :], kw_d[kt])
            nc.sync.dma_start(vw[:, kt, :], vw_d[kt])
        nc.sync.dma_start(pw[:, 0, :], pw_d[0])
        nc.sync.dma_start(pw[:, 1, :], pw_d[1])

        nc.vector.memset(ones_inv[:], 1.0 / DIM)
        nc.vector.memset(ones128f[:], 1.0)
        nc.vector.memset(ones128b[:], 1.0)
        nc.vector.memset(mask0[:], 0.0)
        nc.vector.memset(mask0[0:1, 0:64], 1.0)
        nc.vector.memset(mask1[:], 0.0)
        nc.vector.memset(mask1[0:1, 64:128], 1.0)
        nc.vector.memset(v4[:, :, :, 64:65], 1.0)
        nc.vector.memset(eps1[:], EPS)

        esb = ctx.enter_context(tc.tile_pool(name="esb", bufs=12))
        dbgp = ctx.enter_context(tc.tile_pool(name="dbgp", bufs=1))
        recdp = ctx.enter_context(tc.tile_pool(name="recdp", bufs=3))
        avup = ctx.enter_context(tc.tile_pool(name="avup", bufs=3))
        osbp = ctx.enter_context(tc.tile_pool(name="osbp", bufs=3))
        rbbp = ctx.enter_context(tc.tile_pool(name="rbb", bufs=2))
        ph1 = ExitStack()
        wkps = ph1.enter_context(tc.tile_pool(name="work", bufs=3, space="PSUM"))
        avps1 = ph1.enter_context(tc.tile_pool(name="avrb1", bufs=2, space="PSUM"))
        p1sb = ph1.enter_context(tc.tile_pool(name="p1sb", bufs=4))
        if True:

            def emit_conv(n):
                vt = avps1.tile([1, 512], F32, tag="av")
                for oct_ in range(4):  # oc tiles
                    cps = wkps.tile([P, 512], F32, tag="wk")
                    first = True
                    for kt in range(4):       # kt-major: matches DMA arrival
                        for d in range(4):
                            w = d * 4 + kt
                            di, dj = d // 2, d % 2
                            xv = xt[:, kt, :].rearrange(
                                "p (i a j b) -> p i a j b", i=32, a=2, j=32, b=2)
                            nc.tensor.matmul(
                                cps[:],
                                wc[:, w, oct_ * P:(oct_ + 1) * P],
                                xv[:, 16 * n:16 * (n + 1), di, :, dj],
                                start=first, stop=(kt == 3 and d == 3),
                            )
                            first = False
                    nc.scalar.activation(
                        xrc[:, oct_, n * 512:(n + 1) * 512], cps[:],
                        mybir.ActivationFunctionType.Identity,
                        bias=b4[:, oct_:oct_ + 1])
                    sq = p1sb.tile([P, 512], BF, tag="sq")
                    nc.scalar.activation(
                        sq[:], cps[:],
                        mybir.ActivationFunctionType.Square,
                        bias=b4[:, oct_:oct_ + 1])
                    nc.tensor.matmul(
                        vt[:], ones_inv[:],
                        sq[:], start=(oct_ == 0), stop=(oct_ == 3),
                    )
                return vt

            def emit_ln(n, vt):
                ns = slice(n * 512, (n + 1) * 512)
                # rstd = exp(-0.5*ln(var+eps)) — single ACT table set
                nc.scalar.activation(
                    stdt[0:1, ns], vt[:],
                    mybir.ActivationFunctionType.Ln, bias=eps1[0:1, 0:1])
                nc.scalar.activation(rstd[0:1, ns], stdt[0:1, ns],
                                     mybir.ActivationFunctionType.Exp, scale=-0.5)
                rbc = avps1.tile([P, 512], F32, tag="av")
                nc.tensor.matmul(rbc[:], ones128f[:], rstd[0:1, ns],
                                 start=True, stop=True)
                for kt in range(4):
                    nc.vector.tensor_tensor(
                        xrn[:, kt, ns], xrc[:, kt, ns], rbc[:],
                        mybir.AluOpType.mult)

            def emit_k(n):
                ns = slice(n * 512, (n + 1) * 512)
                for m in range(2):
                    kps = wkps.tile([P, 512], F32, tag="wk")
                    for kt in range(4):
                        nc.tensor.matmul(
                            kps[:],
                            kw[:, kt, m * P:(m + 1) * P],
                            xrn[:, kt, ns],
                            start=(kt == 0), stop=(kt == 3),
                        )
                    nc.scalar.activation(
                        kT2[0:64, 2 * m, ns], kps[0:64, :],
                        mybir.ActivationFunctionType.Identity,
                        bias=kb[0:64, m:m + 1])
                    nc.scalar.activation(
                        kT2[0:64, 2 * m + 1, ns], kps[64:128, :],
                        mybir.ActivationFunctionType.Identity,
                        bias=kb[64:128, m:m + 1])

            def emit_v(n):
                for pt in range(4 * n, 4 * (n + 1)):
                    vps = prps.tile([P, 256], F32, tag="pr")
                    for kt in range(4):
                        nc.tensor.matmul(
                            vps[:],
                            xrn[:, kt, pt * P:(pt + 1) * P],
                            vw[:, kt, :],
                            start=(kt == 0), stop=False,
                        )
                    nc.tensor.matmul(
                        vps[:], ones128b[:], vb[:],
                        start=False, stop=True,
                    )
                    nc.vector.tensor_copy(
                        v4[:, pt, :, 0:64],
                        vps[:].rearrange("p (h d) -> p h d", h=4))

            def emit_q(c):
                cs = slice(c * CHUNK, (c + 1) * CHUNK)
                for m in range(2):
                    qps = qpool[0].tile([P, CHUNK], F32, tag=qpool[1])
                    for kt in range(4):
                        nc.tensor.matmul(
                            qps[:],
                            qw[:, kt, m * P:(m + 1) * P],
                            xt[:, kt, cs],
                            start=(kt == 0), stop=(kt == 3),
                        )
                    nc.vector.tensor_copy(qT2[0:64, 2 * m, cs], qps[0:64, :])
                    nc.vector.tensor_copy(qT2[0:64, 2 * m + 1, cs], qps[64:128, :])

            def emit_proj(pc):
                pcs = slice(pc * CHUNK, (pc + 1) * CHUNK)
                for oct_ in range(4):
                    pps = prps.tile([P, CHUNK], F32, tag="pr")
                    for kt in range(2):
                        nc.tensor.matmul(
                            pps[:],
                            pw[:, kt, oct_ * P:(oct_ + 1) * P],
                            netT[:, kt, pcs],
                            start=(kt == 0), stop=(kt == 1),
                        )
                    osb = osbp.tile([P, CHUNK], F32, tag="osb")
                    nc.vector.tensor_copy(osb[:], pps[:])
                    nc.sync.dma_start(out_d[oct_ * P:(oct_ + 1) * P, pcs], osb[:])

            # ---- software-pipelined attention: one stream of 32 (c, h)
            # tasks; scores+exp of task i overlap av/normalize of task i-1.
            state = {}

            def emit_scores(i):
                c, h = i // 4, i % 4
                cs = slice(c * CHUNK, (c + 1) * CHUNK)
                etiles = []
                for grp in range(4):
                    sps = spsp.tile([P, 2, CHUNK], F32, tag="s")
                    for ti in range(2):
                        t = grp * 2 + ti
                        nc.tensor.matmul(
                            sps[:, ti, :],
                            kT2[0:64, h, t * P:(t + 1) * P],
                            qT2[0:64, h, cs],
                            start=True, stop=True,
                        )
                    ebf = esb.tile([P, 2, CHUNK], BF, tag="e")
                    nc.scalar.activation(
                        ebf[:], sps[:],
                        mybir.ActivationFunctionType.Exp, scale=SCALE)
                    etiles.append(ebf)
                state[i] = etiles

            def emit_av(i):
                c, h = i // 4, i % 4
                cs = slice(c * CHUNK, (c + 1) * CHUNK)
                etiles = state.pop(i)
                avt = avps.tile([P, CHUNK], F32, tag="av")
                for t in range(8):
                    nc.tensor.matmul(
                        avt[0:65, :],
                        v4[:, t, h, :],
                        etiles[t // 2][:, t % 2, :],
                        start=(t == 0), stop=(t == 7),
                    )
                if DEBUG and c == 0 and h == 0:
                    dbga = dbgp.tile([P, CHUNK], F32, tag="dbgt")
                    nc.vector.tensor_copy(dbga[0:65, :], avt[0:65, :])
                    nc.sync.dma_start(dbg_av[0:65, :], dbga[0:65, :])
                    dbge = dbgp.tile([P, 4, CHUNK], F32, tag="dbgt")
                    nc.vector.tensor_copy(dbge[:, 0:2, :], etiles[0][:])
                    nc.vector.tensor_copy(dbge[:, 2:4, :], etiles[1][:])
                    nc.sync.dma_start(dbg_e[:], dbge[:])
                den = recdp.tile([1, CHUNK], F32, tag="den")
                nc.vector.tensor_copy(den[:], avt[64:65, :])
                recf = recdp.tile([1, CHUNK], F32, tag="recf")
                nc.vector.reciprocal_approx_fast(out=recf[:], in_=den[:])
                # broadcast 1/denom across partitions on the idle GPSIMD engine
                rbb = rbbp.tile([P, CHUNK], F32, tag=("rbe" if h % 2 == 0 else "rbo"))
                nc.gpsimd.partition_broadcast(rbb[:], recf[:])
                state[("rbb", i)] = rbb
                if h % 2 == 0:
                    avu_pair = avup.tile([P, CHUNK], BF, tag="avu")
                    nc.vector.tensor_copy(avu_pair[0:64, :], avt[0:64, :])
                    state[("avu", i)] = avu_pair
                else:
                    avu_pair = state.pop(("avu", i - 1))
                    nc.vector.tensor_copy(avu_pair[64:128, :], avt[0:64, :])
                    pr = h // 2
                    rbb_e = state.pop(("rbb", i - 1))
                    rbb_o = state.pop(("rbb", i))
                    if DEBUG and c == 0 and h == 1:
                        dbgu = dbgp.tile([P, CHUNK], F32, tag="dbgt")
                        nc.vector.tensor_copy(dbgu[:], avu_pair[:])
                        nc.sync.dma_start(dbg_avu[:], dbgu[:])
                        dbgr = dbgp.tile([P, CHUNK], F32, tag="dbgt")
                        nc.vector.tensor_copy(dbgr[0:64, :], rbb_e[0:64, :])
                        nc.vector.tensor_copy(dbgr[64:128, :], rbb_o[64:128, :])
                        nc.sync.dma_start(dbg_rb[:], dbgr[:])
                    nc.vector.tensor_tensor(
                        netT[0:64, pr, cs], avu_pair[0:64, :], rbb_e[0:64, :],
                        mybir.AluOpType.mult)
                    nc.vector.tensor_tensor(
                        netT[64:128, pr, cs], avu_pair[64:128, :], rbb_o[64:128, :],
                        mybir.AluOpType.mult)

            # ---- emission: conv, q(c0,c1) fill DMA stalls, LN, kv, then the
            # pipelined task stream with q one chunk ahead
            qpool = (wkps, "wk")
            vt0 = emit_conv(0)
            emit_q(0)
            emit_q(1)
            vt1 = emit_conv(1)
            emit_ln(0, vt0)
            emit_ln(1, vt1)
            emit_k(0)
            emit_k(1)
            ph1.close()
            ph2 = ExitStack()
            spsp = ph2.enter_context(tc.tile_pool(name="sps", bufs=2, space="PSUM"))
            avps = ph2.enter_context(tc.tile_pool(name="avrb", bufs=2, space="PSUM"))
            prps = ph2.enter_context(tc.tile_pool(name="prj", bufs=2, space="PSUM"))
            qpool = (prps, "pr")
            NT = 4 * NCH
            for i in range(NT + 1):
                if i < NT:
                    if i % 4 == 0 and (i // 4) + 2 <= NCH - 1:
                        emit_q((i // 4) + 2)
                    emit_scores(i)
                if i == 0:
                    emit_v(0)
                    emit_v(1)
                if i >= 1:
                    emit_av(i - 1)
                    if (i - 1) % 4 == 3:
                        emit_proj((i - 1) // 4)
            ph2.close()

            if DEBUG:
                dbg_pool = tc.tile_pool(name="dbg", bufs=1)
                with dbg_pool as dp:
                    def dump(dram, sb_ap, nparts):
                        total = 1
                        for s in sb_ap.shape[1:]:
                            total *= s
                        pat_in = {3: "p a b -> p (a b)", 4: "p a b c -> p (a b c)"}
                        flat = (sb_ap.rearrange(pat_in[len(sb_ap.shape)])
                                if len(sb_ap.shape) > 2 else sb_ap)
                        dflat = (dram.rearrange(pat_in[len(dram.shape)])
                                 if len(dram.shape) > 2 else dram)
                        for off in range(0, total, 512):
                            w = min(512, total - off)
                            tt = dp.tile([nparts, 512], F32, tag="dbgt")
                            nc.vector.tensor_copy(tt[:, :w], flat[:, off:off + w])
                            nc.sync.dma_start(dflat[:, off:off + w], tt[:, :w])
                    dump(dbg_xrn, xrn[:], P)
                    dump(dbg_kT2, kT2[:], 64)
                    dump(dbg_qT2, qT2[:], 64)
                    dump(dbg_v4, v4[:], P)
                    dump(dbg_net, netT[:], P)

    nc.finalize()
    return nc


_NC_CACHE = {}


def _get_nc():
    if "nc" not in _NC_CACHE:
        _NC_CACHE["nc"] = build_nc()
    return _NC_CACHE["nc"]


def _prep_core_inputs(x, q_w, kv_w, proj_w, sr_w, sr_b, ln_w, ln_b):
    """Host-side sharding/prep. Returns list of 8 in_maps."""
    bf = NBF
    # conv weights: W4[d, ic, oc] = sr_w[oc, ic, di, dj], d = di*2+dj
    W4 = np.ascontiguousarray(sr_w.transpose(2, 3, 1, 0)).reshape(4, DIM, DIM)
    W4 = W4 - W4.mean(axis=2, keepdims=True)
    wc = W4.reshape(4, 4, P, DIM).reshape(16, P, DIM).astype(bf)
    b4 = (sr_b - sr_b.mean()).reshape(4, P).T.astype(np.float32)
    b4 = np.ascontiguousarray(b4)

    # token gather indices for the strided conv
    ii, jj = np.meshgrid(np.arange(32), np.arange(32), indexing="ij")
    toks = {}
    for di in range(2):
        for dj in range(2):
            toks[di * 2 + dj] = ((2 * ii + di) * 64 + (2 * jj + dj)).reshape(-1)

    kv_w_f = ln_w[:, None] * kv_w          # fold ln weight
    kv_bias = ln_b @ kv_w                   # fold ln bias (1024,)

    in_maps = []
    for c in range(8):
        b, g = c // 2, c % 2
        xT = np.ascontiguousarray(x[b].T)               # (512, 4096)
        xt = xT.reshape(4, P, N).astype(bf)
        qw = np.ascontiguousarray(
            q_w[:, g * 256:(g + 1) * 256]).reshape(4, P, 256).astype(bf)
        kw = np.ascontiguousarray(
            kv_w_f[:, g * 256:(g + 1) * 256]).reshape(4, P, 256).astype(bf)
        vw = np.ascontiguousarray(
            kv_w_f[:, DIM + g * 256:DIM + (g + 1) * 256]).reshape(4, P, 256).astype(bf)
        pwv = np.ascontiguousarray(
            proj_w[g * 256:(g + 1) * 256, :]).reshape(2, P, DIM).astype(bf)
        kbv = np.ascontiguousarray(
            kv_bias[g * 256:(g + 1) * 256].reshape(2, P).T).astype(np.float32)
        vbv = kv_bias[DIM + g * 256:DIM + (g + 1) * 256].reshape(1, 256).astype(bf)
        in_maps.append({
            "xt": xt, "wc": wc, "qw": qw, "kw": kw, "vw": vw,
            "pw": pwv, "b4": b4, "kb": kbv, "vb": vbv,
        })
    return in_maps


def kernel(x, q_w, kv_w, proj_w, proj_b, sr_w, sr_b, ln_w, ln_b, H, W,
           _return_perf=False):
    x = np.asarray(x, dtype=np.float32)
    q_w = np.asarray(q_w, dtype=np.float32)
    kv_w = np.asarray(kv_w, dtype=np.float32)
    proj_w = np.asarray(proj_w, dtype=np.float32)
    proj_b = np.asarray(proj_b, dtype=np.float32)
    sr_w = np.asarray(sr_w, dtype=np.float32)
    sr_b = np.asarray(sr_b, dtype=np.float32)
    ln_w = np.asarray(ln_w, dtype=np.float32)
    ln_b = np.asarray(ln_b, dtype=np.float32)

    in_maps = _prep_core_inputs(x, q_w, kv_w, proj_w, sr_w, sr_b, ln_w, ln_b)
    nc = _get_nc()
    res = run_bass_kernel_spmd(nc, in_maps, core_ids=list(range(8)),
                               trace=_return_perf)
    out = np.empty((BS, N, DIM), dtype=np.float32)
    for b in range(BS):
        partial = res.results[2 * b]["out"] + res.results[2 * b + 1]["out"]
        out[b] = partial.T + proj_b[None, :]
    if _return_perf:
        return out, res
    return out


# revision 80
# speedup vs baseline: 1.0124x; 1.0124x over previous
"""PVT-style spatial-reduction attention on 8 TRN2 NeuronCores.

Sharding: core c -> (batch b = c//2, head-group g = c%2), 4 heads each.
No collectives: each core computes a partial projection output
outT_partial (512, 4096); host sums the two partials per batch.

On-core orientation: features-on-partition, tokens-on-free throughout:
  xT (ch, tok) -> convT (oc, pos) -> LN -> kT (kc, pos) / v (pos, vc)
  qT (qc, tok); scoresT (kv, tok) = kT_h^T-slice @ qT_h; exp on ACT;
  avT (65, tok) via v4 lhsT with ones column (row 64 = softmax denom);
  netT = avT * bcast(1/denom); outT = proj_w_g^T-slice @ netT.
All matmuls bf16 (f32 PSUM) except tiny f32 broadcast matmuls.
"""
import sys as _sys
for _p in ("/opt/trn_rl_repo", "/opt/pypackages"):
    if _p not in _sys.path:
        _sys.path.insert(0, _p)

import numpy as np
import ml_dtypes
from contextlib import ExitStack

import concourse.bass as bass
import concourse.mybir as mybir
import concourse.tile as tile
from concourse import bacc
from concourse.bass_utils import run_bass_kernel_spmd

BF = mybir.dt.bfloat16
F32 = mybir.dt.float32
P = 128
BS, N, DIM, HEADS, HD = 4, 4096, 512, 8, 64
NKV = 1024
SCALE = HD ** -0.5  # 0.125
EPS = 1e-5
NCH = 8          # token chunks of 512
CHUNK = N // NCH  # 512
NBF = np.dtype(ml_dtypes.bfloat16)


DEBUG = False


def build_nc():
    nc = bacc.Bacc()
    xt_d = nc.declare_dram_parameter("xt", (4, P, N), BF, isOutput=False)
    wc_d = nc.declare_dram_parameter("wc", (16, P, DIM), BF, isOutput=False)
    qw_d = nc.declare_dram_parameter("qw", (4, P, 256), BF, isOutput=False)
    kw_d = nc.declare_dram_parameter("kw", (4, P, 256), BF, isOutput=False)
    vw_d = nc.declare_dram_parameter("vw", (4, P, 256), BF, isOutput=False)
    pw_d = nc.declare_dram_parameter("pw", (2, P, DIM), BF, isOutput=False)
    b4_d = nc.declare_dram_parameter("b4", (P, 4), F32, isOutput=False)
    kb_d = nc.declare_dram_parameter("kb", (P, 2), F32, isOutput=False)
    vb_d = nc.declare_dram_parameter("vb", (1, 256), BF, isOutput=False)
    out_d = nc.declare_dram_parameter("out", (DIM, N), F32, isOutput=True)
    if DEBUG:
        dbg_xrn = nc.declare_dram_parameter("dbg_xrn", (P, 4, NKV), F32, isOutput=True)
        dbg_kT2 = nc.declare_dram_parameter("dbg_kT2", (64, 4, NKV), F32, isOutput=True)
        dbg_qT2 = nc.declare_dram_parameter("dbg_qT2", (64, 4, N), F32, isOutput=True)
        dbg_v4 = nc.declare_dram_parameter("dbg_v4", (P, 8, 4, 65), F32, isOutput=True)
        dbg_net = nc.declare_dram_parameter("dbg_net", (P, 2, N), F32, isOutput=True)
        dbg_av = nc.declare_dram_parameter("dbg_av", (P, CHUNK), F32, isOutput=True)
        dbg_avu = nc.declare_dram_parameter("dbg_avu", (P, CHUNK), F32, isOutput=True)
        dbg_rb = nc.declare_dram_parameter("dbg_rb", (P, CHUNK), F32, isOutput=True)
        dbg_e = nc.declare_dram_parameter("dbg_e", (P, 4, CHUNK), F32, isOutput=True)

    with tile.TileContext(nc) as tc, ExitStack() as ctx:
        persist = ctx.enter_context(tc.tile_pool(name="persist", bufs=1))

        # ---- persistent SBUF tensors
        xt = persist.tile([P, 4, N], BF, tag="xt")
        wc = persist.tile([P, 16, DIM], BF, tag="wc")
        qw = persist.tile([P, 4, 256], BF, tag="qw")
        kw = persist.tile([P, 4, 256], BF, tag="kw")
        vw = persist.tile([P, 4, 256], BF, tag="vw")
        pw = persist.tile([P, 2, DIM], BF, tag="pw")
        b4 = persist.tile([P, 4], F32, tag="b4")
        kb = persist.tile([P, 2], F32, tag="kb")
        vb = persist.tile([1, 256], BF, tag="vb")

        xrc = persist.tile([P, 4, NKV], BF, tag="xrc")    # centered conv out
        xrn = persist.tile([P, 4, NKV], BF, tag="xrn")    # LN'd
        qT2 = persist.tile([64, 4, N], BF, tag="qT2")     # per-head q rows
        kT2 = persist.tile([64, 4, NKV], BF, tag="kT2")   # per-head k rows
        v4 = persist.tile([P, 8, 4, 128], BF, tag="v4")   # [ones|pad|v]
        netT = persist.tile([P, 2, N], BF, tag="netT")
        rstd = persist.tile([1, NKV], F32, tag="rstd")
        stdt = persist.tile([1, NKV], F32, tag="stdt")

        ones_inv = persist.tile([P, 1], BF, tag="ones_inv")   # 1/512 column (K=128, M=1)
        ones128f = persist.tile([1, P], F32, tag="ones128f")  # f32 ones row (K=1, M=128)
        eps1 = persist.tile([1, 1], F32, tag="eps1")

        # ---- DMAs: qw first (q fills conv's DMA stalls), xt in quarters,
        # conv weights, then later-stage weights
        nc.sync.dma_start(b4[:], b4_d[:])
        nc.sync.dma_start(kb[:], kb_d[:])
        nc.sync.dma_start(vb[:], vb_d[:])
        for kt in range(4):
            nc.sync.dma_start(qw[:, kt, :], qw_d[kt])
            nc.sync.dma_start(xt[:, kt, 0:1024], xt_d[kt][:, 0:1024])
        for kt in range(4):
            nc.sync.dma_start(xt[:, kt, 1024:2048], xt_d[kt][:, 1024:2048])
        for kt in range(4):
            for d in range(4):
                nc.sync.dma_start(wc[:, d * 4 + kt, :], wc_d[d * 4 + kt])
        for q4 in range(2, 4):
            for kt in range(4):
                nc.sync.dma_start(xt[:, kt, q4 * 1024:(q4 + 1) * 1024],
                                  xt_d[kt][:, q4 * 1024:(q4 + 1) * 1024])
        for kt in range(4):
            nc.sync.dma_start(kw[:, kt, :], kw_d[kt])
            nc.sync.dma_start(vw[:, kt, :], vw_d[kt])
        nc.sync.dma_start(pw[:, 0, :], pw_d[0])
        nc.sync.dma_start(pw[:, 1, :], pw_d[1])

        nc.vector.memset(ones_inv[:], 1.0 / DIM)
        nc.vector.memset(ones128f[:], 1.0)
        nc.vector.memset(v4[:], 0.0)
        nc.vector.memset(v4[:, :, :, 0:1], 1.0)
        nc.vector.memset(eps1[:], EPS)
        vbb = persist.tile([P, 256], BF, tag="vbb")
        nc.gpsimd.partition_broadcast(vbb[:], vb[:])

        esb = ctx.enter_context(tc.tile_pool(name="esb", bufs=12))
        dbgp = ctx.enter_context(tc.tile_pool(name="dbgp", bufs=1))
        recdp = ctx.enter_context(tc.tile_pool(name="recdp", bufs=3))
        avup = ctx.enter_context(tc.tile_pool(name="avup", bufs=3))
        osbp = ctx.enter_context(tc.tile_pool(name="osbp", bufs=3))
        rbbp = ctx.enter_context(tc.tile_pool(name="rbb", bufs=2))
        ph1 = ExitStack()
        wkps = ph1.enter_context(tc.tile_pool(name="work", bufs=3, space="PSUM"))
        avps1 = ph1.enter_context(tc.tile_pool(name="avrb1", bufs=2, space="PSUM"))
        p1sb = ph1.enter_context(tc.tile_pool(name="p1sb", bufs=4))
        if True:

            def emit_conv(n):
                vt = avps1.tile([1, 512], F32, tag="av")
                for oct_ in range(4):  # oc tiles
                    cps = wkps.tile([P, 512], F32, tag="wk")
                    first = True
                    for kt in range(4):       # kt-major: matches DMA arrival
                        for d in range(4):
                            w = d * 4 + kt
                            di, dj = d // 2, d % 2
                            xv = xt[:, kt, :].rearrange(
                                "p (i a j b) -> p i a j b", i=32, a=2, j=32, b=2)
                            nc.tensor.matmul(
                                cps[:],
                                wc[:, w, oct_ * P:(oct_ + 1) * P],
                                xv[:, 16 * n:16 * (n + 1), di, :, dj],
                                start=first, stop=(kt == 3 and d == 3),
                            )
                            first = False
                    nc.scalar.activation(
                        xrc[:, oct_, n * 512:(n + 1) * 512], cps[:],
                        mybir.ActivationFunctionType.Identity,
                        bias=b4[:, oct_:oct_ + 1])
                    sq = p1sb.tile([P, 512], BF, tag="sq")
                    nc.scalar.activation(
                        sq[:], cps[:],
                        mybir.ActivationFunctionType.Square,
                        bias=b4[:, oct_:oct_ + 1])
                    nc.tensor.matmul(
                        vt[:], ones_inv[:],
                        sq[:], start=(oct_ == 0), stop=(oct_ == 3),
                    )
                return vt

            def emit_ln(n, vt):
                ns = slice(n * 512, (n + 1) * 512)
                # rstd = exp(-0.5*ln(var+eps)) — single ACT table set
                nc.scalar.activation(
                    stdt[0:1, ns], vt[:],
                    mybir.ActivationFunctionType.Ln, bias=eps1[0:1, 0:1])
                nc.scalar.activation(rstd[0:1, ns], stdt[0:1, ns],
                                     mybir.ActivationFunctionType.Exp, scale=-0.5)
                rbc = avps1.tile([P, 512], F32, tag="av")
                nc.tensor.matmul(rbc[:], ones128f[:], rstd[0:1, ns],
                                 start=True, stop=True)
                for kt in range(4):
                    nc.vector.tensor_tensor(
                        xrn[:, kt, ns], xrc[:, kt, ns], rbc[:],
                        mybir.AluOpType.mult)

            def emit_k(n):
                ns = slice(n * 512, (n + 1) * 512)
                for m in range(2):
                    kps = wkps.tile([P, 512], F32, tag="wk")
                    for kt in range(4):
                        nc.tensor.matmul(
                            kps[:],
                            kw[:, kt, m * P:(m + 1) * P],
                            xrn[:, kt, ns],
                            start=(kt == 0), stop=(kt == 3),
                        )
                    nc.scalar.activation(
                        kT2[0:64, 2 * m, ns], kps[0:64, :],
                        mybir.ActivationFunctionType.Identity,
                        bias=kb[0:64, m:m + 1])
                    nc.scalar.activation(
                        kT2[0:64, 2 * m + 1, ns], kps[64:128, :],
                        mybir.ActivationFunctionType.Identity,
                        bias=kb[64:128, m:m + 1])

            def emit_v(n):
                for pt in range(4 * n, 4 * (n + 1)):
                    vps = prps.tile([P, 256], F32, tag="pr")
                    for kt in range(4):
                        nc.tensor.matmul(
                            vps[:],
                            xrn[:, kt, pt * P:(pt + 1) * P],
                            vw[:, kt, :],
                            start=(kt == 0), stop=(kt == 3),
                        )
                    nc.vector.scalar_tensor_tensor(
                        v4[:, pt, :, 64:128],
                        vps[:].rearrange("p (h d) -> p h d", h=4),
                        0.0,
                        vbb[:].rearrange("p (h d) -> p h d", h=4),
                        mybir.AluOpType.add,
                        mybir.AluOpType.add)

            def emit_q(c):
                cs = slice(c * CHUNK, (c + 1) * CHUNK)
                for m in range(2):
                    qps = qpool[0].tile([P, CHUNK], F32, tag=qpool[1])
                    for kt in range(4):
                        nc.tensor.matmul(
                            qps[:],
                            qw[:, kt, m * P:(m + 1) * P],
                            xt[:, kt, cs],
                            start=(kt == 0), stop=(kt == 3),
                        )
                    nc.vector.tensor_copy(qT2[0:64, 2 * m, cs], qps[0:64, :])
                    nc.vector.tensor_copy(qT2[0:64, 2 * m + 1, cs], qps[64:128, :])

            def emit_proj(pc):
                pcs = slice(pc * CHUNK, (pc + 1) * CHUNK)
                for oct_ in range(4):
                    pps = prps.tile([P, CHUNK], F32, tag="pr")
                    for kt in range(2):
                        nc.tensor.matmul(
                            pps[:],
                            pw[:, kt, oct_ * P:(oct_ + 1) * P],
                            netT[:, kt, pcs],
                            start=(kt == 0), stop=(kt == 1),
                        )
                    osb = osbp.tile([P, CHUNK], F32, tag="osb")
                    nc.vector.tensor_copy(osb[:], pps[:])
                    nc.sync.dma_start(out_d[oct_ * P:(oct_ + 1) * P, pcs], osb[:])

            # ---- software-pipelined attention: one stream of 32 (c, h)
            # tasks; scores+exp of task i overlap av/normalize of task i-1.
            state = {}

            def emit_scores(i):
                c, h = i // 4, i % 4
                cs = slice(c * CHUNK, (c + 1) * CHUNK)
                etiles = []
                for grp in range(4):
                    sps = spsp.tile([P, 2, CHUNK], F32, tag="s")
                    for ti in range(2):
                        t = grp * 2 + ti
                        nc.tensor.matmul(
                            sps[:, ti, :],
                            kT2[0:64, h, t * P:(t + 1) * P],
                            qT2[0:64, h, cs],
                            start=True, stop=True,
                        )
                    ebf = esb.tile([P, 2, CHUNK], BF, tag="e")
                    nc.scalar.activation(
                        ebf[:], sps[:],
                        mybir.ActivationFunctionType.Exp, scale=SCALE)
                    etiles.append(ebf)
                state[i] = etiles

            def emit_av(i):
                c, h = i // 4, i % 4
                cs = slice(c * CHUNK, (c + 1) * CHUNK)
                etiles = state.pop(i)
                avt = avps.tile([P, CHUNK], F32, tag="av")
                for t in range(8):
                    nc.tensor.matmul(
                        avt[0:128, :],
                        v4[:, t, h, :],
                        etiles[t // 2][:, t % 2, :],
                        start=(t == 0), stop=(t == 7),
                    )
                if DEBUG and c == 0 and h == 0:
                    dbga = dbgp.tile([P, CHUNK], F32, tag="dbgt")
                    nc.vector.tensor_copy(dbga[0:128, :], avt[0:128, :])
                    nc.sync.dma_start(dbg_av[0:128, :], dbga[0:128, :])
                    dbge = dbgp.tile([P, 4, CHUNK], F32, tag="dbgt")
                    nc.vector.tensor_copy(dbge[:, 0:2, :], etiles[0][:])
                    nc.vector.tensor_copy(dbge[:, 2:4, :], etiles[1][:])
                    nc.sync.dma_start(dbg_e[:], dbge[:])
                recf = recdp.tile([1, CHUNK], F32, tag="recf")
                nc.vector.reciprocal_approx_fast(out=recf[:], in_=avt[0:1, :])
                # broadcast 1/denom across partitions on the idle GPSIMD engine
                rbb = rbbp.tile([P, CHUNK], F32, tag=("rbe" if h % 2 == 0 else "rbo"))
                nc.gpsimd.partition_broadcast(rbb[:], recf[:])
                state[("rbb", i)] = rbb
                if h % 2 == 0:
                    avu_pair = avup.tile([P, CHUNK], BF, tag="avu")
                    nc.vector.tensor_copy(avu_pair[0:64, :], avt[64:128, :])
                    state[("avu", i)] = avu_pair
                else:
                    avu_pair = state.pop(("avu", i - 1))
                    nc.vector.tensor_copy(avu_pair[64:128, :], avt[64:128, :])
                    pr = h // 2
                    rbb_e = state.pop(("rbb", i - 1))
                    rbb_o = state.pop(("rbb", i))
                    if DEBUG and c == 0 and h == 1:
                        dbgu = dbgp.tile([P, CHUNK], F32, tag="dbgt")
                        nc.vector.tensor_copy(dbgu[:], avu_pair[:])
                        nc.sync.dma_start(dbg_avu[:], dbgu[:])
                        dbgr = dbgp.tile([P, CHUNK], F32, tag="dbgt")
                        nc.vector.tensor_copy(dbgr[0:64, :], rbb_e[0:64, :])
                        nc.vector.tensor_copy(dbgr[64:128, :], rbb_o[64:128, :])
                        nc.sync.dma_start(dbg_rb[:], dbgr[:])
                    nc.vector.tensor_tensor(
                        netT[0:64, pr, cs], avu_pair[0:64, :], rbb_e[0:64, :],
                        mybir.AluOpType.mult)
                    nc.vector.tensor_tensor(
                        netT[64:128, pr, cs], avu_pair[64:128, :], rbb_o[64:128, :],
                        mybir.AluOpType.mult)

            # ---- emission: conv, q(c0,c1) fill DMA stalls, LN, kv, then the
            # pipelined task stream with q one chunk ahead
            qpool = (wkps, "wk")
            vt0 = emit_conv(0)
            emit_q(0)
            emit_q(1)
            vt1 = emit_conv(1)
            emit_ln(0, vt0)
            emit_ln(1, vt1)
            emit_k(0)
            emit_k(1)
            ph1.close()
            ph2 = ExitStack()
            spsp = ph2.enter_context(tc.tile_pool(name="sps", bufs=2, space="PSUM"))
            avps = ph2.enter_context(tc.tile_pool(name="avrb", bufs=2, space="PSUM"))
            prps = ph2.enter_context(tc.tile_pool(name="prj", bufs=2, space="PSUM"))
            qpool = (prps, "pr")
            NT = 4 * NCH
            for i in range(NT + 1):
                if i < NT:
                    if i % 4 == 0 and (i // 4) + 2 <= NCH - 1:
                        emit_q((i // 4) + 2)
                    emit_scores(i)
                if i == 0:
                    emit_v(0)
                    emit_v(1)
                if i >= 1:
                    emit_av(i - 1)
                    if (i - 1) % 4 == 3:
                        emit_proj((i - 1) // 4)
            ph2.close()

            if DEBUG:
                dbg_pool = tc.tile_pool(name="dbg", bufs=1)
                with dbg_pool as dp:
                    def dump(dram, sb_ap, nparts):
                        total = 1
                        for s in sb_ap.shape[1:]:
                            total *= s
                        pat_in = {3: "p a b -> p (a b)", 4: "p a b c -> p (a b c)"}
                        flat = (sb_ap.rearrange(pat_in[len(sb_ap.shape)])
                                if len(sb_ap.shape) > 2 else sb_ap)
                        dflat = (dram.rearrange(pat_in[len(dram.shape)])
                                 if len(dram.shape) > 2 else dram)
                        for off in range(0, total, 512):
                            w = min(512, total - off)
                            tt = dp.tile([nparts, 512], F32, tag="dbgt")
                            nc.vector.tensor_copy(tt[:, :w], flat[:, off:off + w])
                            nc.sync.dma_start(dflat[:, off:off + w], tt[:, :w])
                    dump(dbg_xrn, xrn[:], P)
                    dump(dbg_kT2, kT2[:], 64)
                    dump(dbg_qT2, qT2[:], 64)
                    dump(dbg_v4, v4[:], P)
                    dump(dbg_net, netT[:], P)

    nc.finalize()
    return nc


_NC_CACHE = {}


def _get_nc():
    if "nc" not in _NC_CACHE:
        _NC_CACHE["nc"] = build_nc()
    return _NC_CACHE["nc"]


def _prep_core_inputs(x, q_w, kv_w, proj_w, sr_w, sr_b, ln_w, ln_b):
    """Host-side sharding/prep. Returns list of 8 in_maps."""
    bf = NBF
    # conv weights: W4[d, ic, oc] = sr_w[oc, ic, di, dj], d = di*2+dj
    W4 = np.ascontiguousarray(sr_w.transpose(2, 3, 1, 0)).reshape(4, DIM, DIM)
    W4 = W4 - W4.mean(axis=2, keepdims=True)
    wc = W4.reshape(4, 4, P, DIM).reshape(16, P, DIM).astype(bf)
    b4 = (sr_b - sr_b.mean()).reshape(4, P).T.astype(np.float32)
    b4 = np.ascontiguousarray(b4)

    # token gather indices for the strided conv
    ii, jj = np.meshgrid(np.arange(32), np.arange(32), indexing="ij")
    toks = {}
    for di in range(2):
        for dj in range(2):
            toks[di * 2 + dj] = ((2 * ii + di) * 64 + (2 * jj + dj)).reshape(-1)

    kv_w_f = ln_w[:, None] * kv_w          # fold ln weight
    kv_bias = ln_b @ kv_w                   # fold ln bias (1024,)

    in_maps = []
    for c in range(8):
        b, g = c // 2, c % 2
        xT = np.ascontiguousarray(x[b].T)               # (512, 4096)
        xt = xT.reshape(4, P, N).astype(bf)
        qw = np.ascontiguousarray(
            q_w[:, g * 256:(g + 1) * 256]).reshape(4, P, 256).astype(bf)
        kw = np.ascontiguousarray(
            kv_w_f[:, g * 256:(g + 1) * 256]).reshape(4, P, 256).astype(bf)
        vw = np.ascontiguousarray(
            kv_w_f[:, DIM + g * 256:DIM + (g + 1) * 256]).reshape(4, P, 256).astype(bf)
        pwv = np.ascontiguousarray(
            proj_w[g * 256:(g + 1) * 256, :]).reshape(2, P, DIM).astype(bf)
        kbv = np.ascontiguousarray(
            kv_bias[g * 256:(g + 1) * 256].reshape(2, P).T).astype(np.float32)
        vbv = kv_bias[DIM + g * 256:DIM + (g + 1) * 256].reshape(1, 256).astype(bf)
        in_maps.append({
            "xt": xt, "wc": wc, "qw": qw, "kw": kw, "vw": vw,
            "pw": pwv, "b4": b4, "kb": kbv, "vb": vbv,
        })
    return in_maps


def kernel(x, q_w, kv_w, proj_w, proj_b, sr_w, sr_b, ln_w, ln_b, H, W,
           _return_perf=False):
    x = np.asarray(x, dtype=np.float32)
    q_w = np.asarray(q_w, dtype=np.float32)
    kv_w = np.asarray(kv_w, dtype=np.float32)
    proj_w = np.asarray(proj_w, dtype=np.float32)
    proj_b = np.asarray(proj_b, dtype=np.float32)
    sr_w = np.asarray(sr_w, dtype=np.float32)
    sr_b = np.asarray(sr_b, dtype=np.float32)
    ln_w = np.asarray(ln_w, dtype=np.float32)
    ln_b = np.asarray(ln_b, dtype=np.float32)

    in_maps = _prep_core_inputs(x, q_w, kv_w, proj_w, sr_w, sr_b, ln_w, ln_b)
    nc = _get_nc()
    res = run_bass_kernel_spmd(nc, in_maps, core_ids=list(range(8)),
                               trace=_return_perf)
    out = np.empty((BS, N, DIM), dtype=np.float32)
    for b in range(BS):
        partial = res.results[2 * b]["out"] + res.results[2 * b + 1]["out"]
        out[b] = partial.T + proj_b[None, :]
    if _return_perf:
        return out, res
    return out


# revision 88
# speedup vs baseline: 1.0290x; 1.0164x over previous
"""PVT-style spatial-reduction attention on 8 TRN2 NeuronCores.

Sharding: core c -> (batch b = c//2, head-group g = c%2), 4 heads each.
No collectives: each core computes a partial projection output
outT_partial (512, 4096); host sums the two partials per batch.

On-core orientation: features-on-partition, tokens-on-free throughout:
  xT (ch, tok) -> convT (oc, pos) -> LN -> kT (kc, pos) / v (pos, vc)
  qT (qc, tok); scoresT (kv, tok) = kT_h^T-slice @ qT_h; exp on ACT;
  avT (65, tok) via v4 lhsT with ones column (row 64 = softmax denom);
  netT = avT * bcast(1/denom); outT = proj_w_g^T-slice @ netT.
All matmuls bf16 (f32 PSUM) except tiny f32 broadcast matmuls.
"""
import sys as _sys
for _p in ("/opt/trn_rl_repo", "/opt/pypackages"):
    if _p not in _sys.path:
        _sys.path.insert(0, _p)

import numpy as np
import ml_dtypes
from contextlib import ExitStack

import concourse.bass as bass
import concourse.mybir as mybir
import concourse.tile as tile
from concourse import bacc
from concourse.bass_utils import run_bass_kernel_spmd

BF = mybir.dt.bfloat16
F32 = mybir.dt.float32
P = 128
BS, N, DIM, HEADS, HD = 4, 4096, 512, 8, 64
NKV = 1024
SCALE = HD ** -0.5  # 0.125
EPS = 1e-5
NCH = 8          # token chunks of 512
CHUNK = N // NCH  # 512
NBF = np.dtype(ml_dtypes.bfloat16)


DEBUG = False


def build_nc():
    nc = bacc.Bacc()
    xt_d = nc.declare_dram_parameter("xt", (4, P, N), BF, isOutput=False)
    wc_d = nc.declare_dram_parameter("wc", (16, P, DIM), BF, isOutput=False)
    qw_d = nc.declare_dram_parameter("qw", (4, P, 256), BF, isOutput=False)
    kw_d = nc.declare_dram_parameter("kw", (4, P, 256), BF, isOutput=False)
    vw_d = nc.declare_dram_parameter("vw", (4, P, 256), BF, isOutput=False)
    pw_d = nc.declare_dram_parameter("pw", (2, P, DIM), BF, isOutput=False)
    b4_d = nc.declare_dram_parameter("b4", (P, 4), F32, isOutput=False)
    kb_d = nc.declare_dram_parameter("kb", (P, 2), F32, isOutput=False)
    vb_d = nc.declare_dram_parameter("vb", (1, 256), BF, isOutput=False)
    out_d = nc.declare_dram_parameter("out", (DIM, N), F32, isOutput=True)
    if DEBUG:
        dbg_xrn = nc.declare_dram_parameter("dbg_xrn", (P, 4, NKV), F32, isOutput=True)
        dbg_kT2 = nc.declare_dram_parameter("dbg_kT2", (64, 4, NKV), F32, isOutput=True)
        dbg_qT2 = nc.declare_dram_parameter("dbg_qT2", (64, 4, N), F32, isOutput=True)
        dbg_v4 = nc.declare_dram_parameter("dbg_v4", (P, 8, 4, 65), F32, isOutput=True)
        dbg_net = nc.declare_dram_parameter("dbg_net", (P, 2, N), F32, isOutput=True)
        dbg_av = nc.declare_dram_parameter("dbg_av", (P, CHUNK), F32, isOutput=True)
        dbg_avu = nc.declare_dram_parameter("dbg_avu", (P, CHUNK), F32, isOutput=True)
        dbg_rb = nc.declare_dram_parameter("dbg_rb", (P, CHUNK), F32, isOutput=True)
        dbg_e = nc.declare_dram_parameter("dbg_e", (P, 4, CHUNK), F32, isOutput=True)

    with tile.TileContext(nc) as tc, ExitStack() as ctx:
        persist = ctx.enter_context(tc.tile_pool(name="persist", bufs=1))

        # ---- persistent SBUF tensors
        xt = persist.tile([P, 4, N], BF, tag="xt")
        wc = persist.tile([P, 16, DIM], BF, tag="wc")
        qw = persist.tile([P, 4, 256], BF, tag="qw")
        kw = persist.tile([P, 4, 256], BF, tag="kw")
        vw = persist.tile([P, 4, 256], BF, tag="vw")
        pw = persist.tile([P, 2, DIM], BF, tag="pw")
        b4 = persist.tile([P, 4], F32, tag="b4")
        kb = persist.tile([P, 2], F32, tag="kb")
        vb = persist.tile([1, 256], BF, tag="vb")

        xrc = persist.tile([P, 4, NKV], BF, tag="xrc")    # centered conv out
        xrn = persist.tile([P, 4, NKV], BF, tag="xrn")    # LN'd
        qT2 = persist.tile([64, 4, N], BF, tag="qT2")     # per-head q rows
        kT2 = persist.tile([64, 4, NKV], BF, tag="kT2")   # per-head k rows
        v4 = persist.tile([P, 8, 4, 128], BF, tag="v4")   # [ones|pad|v]
        netT = persist.tile([P, 2, N], BF, tag="netT")
        rstd = persist.tile([1, NKV], F32, tag="rstd")
        stdt = persist.tile([1, NKV], F32, tag="stdt")

        ones_inv = persist.tile([P, 1], BF, tag="ones_inv")   # 1/512 column (K=128, M=1)
        ones128f = persist.tile([1, P], F32, tag="ones128f")  # f32 ones row (K=1, M=128)
        eps1 = persist.tile([1, 1], F32, tag="eps1")

        # ---- DMAs: qw first (q fills conv's DMA stalls), xt in quarters,
        # conv weights, then later-stage weights
        nc.sync.dma_start(b4[:], b4_d[:])
        nc.sync.dma_start(kb[:], kb_d[:])
        nc.sync.dma_start(vb[:], vb_d[:])
        for kt in range(4):
            nc.sync.dma_start(qw[:, kt, :], qw_d[kt])
            nc.sync.dma_start(xt[:, kt, 0:1024], xt_d[kt][:, 0:1024])
        for kt in range(4):
            nc.sync.dma_start(xt[:, kt, 1024:2048], xt_d[kt][:, 1024:2048])
        for kt in range(4):
            for d in range(4):
                nc.sync.dma_start(wc[:, d * 4 + kt, :], wc_d[d * 4 + kt])
        for q4 in range(2, 4):
            for kt in range(4):
                nc.sync.dma_start(xt[:, kt, q4 * 1024:(q4 + 1) * 1024],
                                  xt_d[kt][:, q4 * 1024:(q4 + 1) * 1024])
        for kt in range(4):
            nc.sync.dma_start(kw[:, kt, :], kw_d[kt])
            nc.sync.dma_start(vw[:, kt, :], vw_d[kt])
        nc.sync.dma_start(pw[:, 0, :], pw_d[0])
        nc.sync.dma_start(pw[:, 1, :], pw_d[1])

        nc.vector.memset(ones_inv[:], 1.0 / DIM)
        nc.vector.memset(ones128f[:], 1.0)
        nc.vector.memset(v4[:], 0.0)
        nc.vector.memset(v4[:, :, :, 0:1], 1.0)
        nc.vector.memset(eps1[:], EPS)
        vbb = persist.tile([P, 256], BF, tag="vbb")
        nc.gpsimd.partition_broadcast(vbb[:], vb[:])

        esb = ctx.enter_context(tc.tile_pool(name="esb", bufs=12))
        dbgp = ctx.enter_context(tc.tile_pool(name="dbgp", bufs=1))
        recdp = ctx.enter_context(tc.tile_pool(name="recdp", bufs=3))
        avup = ctx.enter_context(tc.tile_pool(name="avup", bufs=3))
        osbp = ctx.enter_context(tc.tile_pool(name="osbp", bufs=4))
        rbbp = ctx.enter_context(tc.tile_pool(name="rbb", bufs=4))
        ph1 = ExitStack()
        wkps = ph1.enter_context(tc.tile_pool(name="work", bufs=4, space="PSUM"))
        avps1 = ph1.enter_context(tc.tile_pool(name="avrb1", bufs=2, space="PSUM"))
        p1sb = ph1.enter_context(tc.tile_pool(name="p1sb", bufs=4))
        if True:

            def emit_conv(n):
                vt = avps1.tile([1, 512], F32, tag="av")
                for oct_ in range(4):  # oc tiles
                    cps = wkps.tile([P, 512], F32, tag="wk")
                    first = True
                    for kt in range(4):       # kt-major: matches DMA arrival
                        for d in range(4):
                            w = d * 4 + kt
                            di, dj = d // 2, d % 2
                            xv = xt[:, kt, :].rearrange(
                                "p (i a j b) -> p i a j b", i=32, a=2, j=32, b=2)
                            nc.tensor.matmul(
                                cps[:],
                                wc[:, w, oct_ * P:(oct_ + 1) * P],
                                xv[:, 16 * n:16 * (n + 1), di, :, dj],
                                start=first, stop=(kt == 3 and d == 3),
                            )
                            first = False
                    nc.scalar.activation(
                        xrc[:, oct_, n * 512:(n + 1) * 512], cps[:],
                        mybir.ActivationFunctionType.Identity,
                        bias=b4[:, oct_:oct_ + 1])
                    sq = p1sb.tile([P, 512], BF, tag="sq")
                    nc.scalar.activation(
                        sq[:], cps[:],
                        mybir.ActivationFunctionType.Square,
                        bias=b4[:, oct_:oct_ + 1])
                    nc.tensor.matmul(
                        vt[:], ones_inv[:],
                        sq[:], start=(oct_ == 0), stop=(oct_ == 3),
                    )
                return vt

            def emit_ln(n, vt):
                ns = slice(n * 512, (n + 1) * 512)
                # rstd = exp(-0.5*ln(var+eps)) — single ACT table set
                nc.scalar.activation(
                    stdt[0:1, ns], vt[:],
                    mybir.ActivationFunctionType.Ln, bias=eps1[0:1, 0:1])
                nc.scalar.activation(rstd[0:1, ns], stdt[0:1, ns],
                                     mybir.ActivationFunctionType.Exp, scale=-0.5)
                rbc = avps1.tile([P, 512], F32, tag="av")
                nc.tensor.matmul(rbc[:], ones128f[:], rstd[0:1, ns],
                                 start=True, stop=True)
                for kt in range(4):
                    nc.vector.tensor_tensor(
                        xrn[:, kt, ns], xrc[:, kt, ns], rbc[:],
                        mybir.AluOpType.mult)

            def emit_k(n):
                ns = slice(n * 512, (n + 1) * 512)
                for m in range(2):
                    kps = wkps.tile([P, 512], F32, tag="wk")
                    for kt in range(4):
                        nc.tensor.matmul(
                            kps[:],
                            kw[:, kt, m * P:(m + 1) * P],
                            xrn[:, kt, ns],
                            start=(kt == 0), stop=(kt == 3),
                        )
                    nc.scalar.activation(
                        kT2[0:64, 2 * m, ns], kps[0:64, :],
                        mybir.ActivationFunctionType.Identity,
                        bias=kb[0:64, m:m + 1])
                    nc.scalar.activation(
                        kT2[0:64, 2 * m + 1, ns], kps[64:128, :],
                        mybir.ActivationFunctionType.Identity,
                        bias=kb[64:128, m:m + 1])

            def emit_v(n):
                for pt in range(4 * n, 4 * (n + 1)):
                    vps = prps.tile([P, 256], F32, tag="pr")
                    for kt in range(4):
                        nc.tensor.matmul(
                            vps[:],
                            xrn[:, kt, pt * P:(pt + 1) * P],
                            vw[:, kt, :],
                            start=(kt == 0), stop=(kt == 3),
                        )
                    nc.vector.scalar_tensor_tensor(
                        v4[:, pt, :, 64:128],
                        vps[:].rearrange("p (h d) -> p h d", h=4),
                        0.0,
                        vbb[:].rearrange("p (h d) -> p h d", h=4),
                        mybir.AluOpType.add,
                        mybir.AluOpType.add)

            def emit_q(c):
                cs = slice(c * CHUNK, (c + 1) * CHUNK)
                for m in range(2):
                    qps = qpool[0].tile([P, CHUNK], F32, tag=qpool[1])
                    for kt in range(4):
                        nc.tensor.matmul(
                            qps[:],
                            qw[:, kt, m * P:(m + 1) * P],
                            xt[:, kt, cs],
                            start=(kt == 0), stop=(kt == 3),
                        )
                    nc.vector.tensor_copy(qT2[0:64, 2 * m, cs], qps[0:64, :])
                    nc.vector.tensor_copy(qT2[0:64, 2 * m + 1, cs], qps[64:128, :])

            def emit_proj(pc):
                pcs = slice(pc * CHUNK, (pc + 1) * CHUNK)
                for oct_ in range(4):
                    pps = prps.tile([P, CHUNK], F32, tag="pr")
                    for kt in range(2):
                        nc.tensor.matmul(
                            pps[:],
                            pw[:, kt, oct_ * P:(oct_ + 1) * P],
                            netT[:, kt, pcs],
                            start=(kt == 0), stop=(kt == 1),
                        )
                    osb = osbp.tile([P, CHUNK], F32, tag="osb")
                    nc.vector.tensor_copy(osb[:], pps[:])
                    nc.sync.dma_start(out_d[oct_ * P:(oct_ + 1) * P, pcs], osb[:])

            # ---- software-pipelined attention: one stream of 32 (c, h)
            # tasks; scores+exp of task i overlap av/normalize of task i-1.
            state = {}

            def emit_scores(i):
                c, h = i // 4, i % 4
                cs = slice(c * CHUNK, (c + 1) * CHUNK)
                etiles = []
                for grp in range(4):
                    sps = spsp.tile([P, 2, CHUNK], F32, tag="s")
                    for ti in range(2):
                        t = grp * 2 + ti
                        nc.tensor.matmul(
                            sps[:, ti, :],
                            kT2[0:64, h, t * P:(t + 1) * P],
                            qT2[0:64, h, cs],
                            start=True, stop=True,
                        )
                    ebf = esb.tile([P, 2, CHUNK], BF, tag="e")
                    nc.scalar.activation(
                        ebf[:], sps[:],
                        mybir.ActivationFunctionType.Exp, scale=SCALE)
                    etiles.append(ebf)
                state[i] = etiles

            def emit_av(i):
                c, h = i // 4, i % 4
                cs = slice(c * CHUNK, (c + 1) * CHUNK)
                etiles = state.pop(i)
                avt = avps.tile([P, CHUNK], F32, tag="av")
                for t in range(8):
                    nc.tensor.matmul(
                        avt[0:128, :],
                        v4[:, t, h, :],
                        etiles[t // 2][:, t % 2, :],
                        start=(t == 0), stop=(t == 7),
                    )
                if DEBUG and c == 0 and h == 0:
                    dbga = dbgp.tile([P, CHUNK], F32, tag="dbgt")
                    nc.vector.tensor_copy(dbga[0:128, :], avt[0:128, :])
                    nc.sync.dma_start(dbg_av[0:128, :], dbga[0:128, :])
                    dbge = dbgp.tile([P, 4, CHUNK], F32, tag="dbgt")
                    nc.vector.tensor_copy(dbge[:, 0:2, :], etiles[0][:])
                    nc.vector.tensor_copy(dbge[:, 2:4, :], etiles[1][:])
                    nc.sync.dma_start(dbg_e[:], dbge[:])
                recf = recdp.tile([1, CHUNK], F32, tag="recf")
                nc.vector.reciprocal_approx_fast(out=recf[:], in_=avt[0:1, :])
                # broadcast 1/denom across partitions on the idle GPSIMD engine
                rbb = rbbp.tile([P, CHUNK], F32, tag=("rbe" if h % 2 == 0 else "rbo"))
                nc.gpsimd.partition_broadcast(rbb[:], recf[:])
                state[("rbb", i)] = rbb
                if h % 2 == 0:
                    avu_pair = avup.tile([P, CHUNK], BF, tag="avu")
                    nc.vector.tensor_copy(avu_pair[0:64, :], avt[64:128, :])
                    state[("avu", i)] = avu_pair
                else:
                    avu_pair = state.pop(("avu", i - 1))
                    nc.vector.tensor_copy(avu_pair[64:128, :], avt[64:128, :])
                    pr = h // 2
                    rbb_e = state.pop(("rbb", i - 1))
                    rbb_o = state.pop(("rbb", i))
                    if DEBUG and c == 0 and h == 1:
                        dbgu = dbgp.tile([P, CHUNK], F32, tag="dbgt")
                        nc.vector.tensor_copy(dbgu[:], avu_pair[:])
                        nc.sync.dma_start(dbg_avu[:], dbgu[:])
                        dbgr = dbgp.tile([P, CHUNK], F32, tag="dbgt")
                        nc.vector.tensor_copy(dbgr[0:64, :], rbb_e[0:64, :])
                        nc.vector.tensor_copy(dbgr[64:128, :], rbb_o[64:128, :])
                        nc.sync.dma_start(dbg_rb[:], dbgr[:])
                    nc.vector.tensor_tensor(
                        netT[0:64, pr, cs], avu_pair[0:64, :], rbb_e[0:64, :],
                        mybir.AluOpType.mult)
                    nc.vector.tensor_tensor(
                        netT[64:128, pr, cs], avu_pair[64:128, :], rbb_o[64:128, :],
                        mybir.AluOpType.mult)

            # ---- emission: conv, q(c0,c1) fill DMA stalls, LN, kv, then the
            # pipelined task stream with q one chunk ahead
            qpool = (wkps, "wk")
            vt0 = emit_conv(0)
            emit_q(0)
            emit_q(1)
            vt1 = emit_conv(1)
            emit_ln(0, vt0)
            emit_ln(1, vt1)
            emit_k(0)
            emit_k(1)
            ph1.close()
            ph2 = ExitStack()
            spsp = ph2.enter_context(tc.tile_pool(name="sps", bufs=2, space="PSUM"))
            avps = ph2.enter_context(tc.tile_pool(name="avrb", bufs=2, space="PSUM"))
            prps = ph2.enter_context(tc.tile_pool(name="prj", bufs=2, space="PSUM"))
            qpool = (prps, "pr")
            NT = 4 * NCH
            for i in range(NT + 1):
                if i < NT:
                    if i % 4 == 0 and (i // 4) + 2 <= NCH - 1:
                        emit_q((i // 4) + 2)
                    emit_scores(i)
                if i == 0:
                    emit_v(0)
                    emit_v(1)
                if i >= 1:
                    emit_av(i - 1)
                    if (i - 1) % 4 == 3:
                        emit_proj((i - 1) // 4)
            ph2.close()

            if DEBUG:
                dbg_pool = tc.tile_pool(name="dbg", bufs=1)
                with dbg_pool as dp:
                    def dump(dram, sb_ap, nparts):
                        total = 1
                        for s in sb_ap.shape[1:]:
                            total *= s
                        pat_in = {3: "p a b -> p (a b)", 4: "p a b c -> p (a b c)"}
                        flat = (sb_ap.rearrange(pat_in[len(sb_ap.shape)])
                                if len(sb_ap.shape) > 2 else sb_ap)
                        dflat = (dram.rearrange(pat_in[len(dram.shape)])
                                 if len(dram.shape) > 2 else dram)
                        for off in range(0, total, 512):
                            w = min(512, total - off)
                            tt = dp.tile([nparts, 512], F32, tag="dbgt")
                            nc.vector.tensor_copy(tt[:, :w], flat[:, off:off + w])
                            nc.sync.dma_start(dflat[:, off:off + w], tt[:, :w])
                    dump(dbg_xrn, xrn[:], P)
                    dump(dbg_kT2, kT2[:], 64)
                    dump(dbg_qT2, qT2[:], 64)
                    dump(dbg_v4, v4[:], P)
                    dump(dbg_net, netT[:], P)

    nc.finalize()
    return nc


_NC_CACHE = {}


def _get_nc():
    if "nc" not in _NC_CACHE:
        _NC_CACHE["nc"] = build_nc()
    return _NC_CACHE["nc"]


def _prep_core_inputs(x, q_w, kv_w, proj_w, sr_w, sr_b, ln_w, ln_b):
    """Host-side sharding/prep. Returns list of 8 in_maps."""
    bf = NBF
    # conv weights: W4[d, ic, oc] = sr_w[oc, ic, di, dj], d = di*2+dj
    W4 = np.ascontiguousarray(sr_w.transpose(2, 3, 1, 0)).reshape(4, DIM, DIM)
    W4 = W4 - W4.mean(axis=2, keepdims=True)
    wc = W4.reshape(4, 4, P, DIM).reshape(16, P, DIM).astype(bf)
    b4 = (sr_b - sr_b.mean()).reshape(4, P).T.astype(np.float32)
    b4 = np.ascontiguousarray(b4)

    # token gather indices for the strided conv
    ii, jj = np.meshgrid(np.arange(32), np.arange(32), indexing="ij")
    toks = {}
    for di in range(2):
        for dj in range(2):
            toks[di * 2 + dj] = ((2 * ii + di) * 64 + (2 * jj + dj)).reshape(-1)

    kv_w_f = ln_w[:, None] * kv_w          # fold ln weight
    kv_bias = ln_b @ kv_w                   # fold ln bias (1024,)

    in_maps = []
    for c in range(8):
        b, g = c // 2, c % 2
        xT = np.ascontiguousarray(x[b].T)               # (512, 4096)
        xt = xT.reshape(4, P, N).astype(bf)
        qw = np.ascontiguousarray(
            q_w[:, g * 256:(g + 1) * 256]).reshape(4, P, 256).astype(bf)
        kw = np.ascontiguousarray(
            kv_w_f[:, g * 256:(g + 1) * 256]).reshape(4, P, 256).astype(bf)
        vw = np.ascontiguousarray(
            kv_w_f[:, DIM + g * 256:DIM + (g + 1) * 256]).reshape(4, P, 256).astype(bf)
        pwv = np.ascontiguousarray(
            proj_w[g * 256:(g + 1) * 256, :]).reshape(2, P, DIM).astype(bf)
        kbv = np.ascontiguousarray(
            kv_bias[g * 256:(g + 1) * 256].reshape(2, P).T).astype(np.float32)
        vbv = kv_bias[DIM + g * 256:DIM + (g + 1) * 256].reshape(1, 256).astype(bf)
        in_maps.append({
            "xt": xt, "wc": wc, "qw": qw, "kw": kw, "vw": vw,
            "pw": pwv, "b4": b4, "kb": kbv, "vb": vbv,
        })
    return in_maps


def kernel(x, q_w, kv_w, proj_w, proj_b, sr_w, sr_b, ln_w, ln_b, H, W,
           _return_perf=False):
    x = np.asarray(x, dtype=np.float32)
    q_w = np.asarray(q_w, dtype=np.float32)
    kv_w = np.asarray(kv_w, dtype=np.float32)
    proj_w = np.asarray(proj_w, dtype=np.float32)
    proj_b = np.asarray(proj_b, dtype=np.float32)
    sr_w = np.asarray(sr_w, dtype=np.float32)
    sr_b = np.asarray(sr_b, dtype=np.float32)
    ln_w = np.asarray(ln_w, dtype=np.float32)
    ln_b = np.asarray(ln_b, dtype=np.float32)

    in_maps = _prep_core_inputs(x, q_w, kv_w, proj_w, sr_w, sr_b, ln_w, ln_b)
    nc = _get_nc()
    res = run_bass_kernel_spmd(nc, in_maps, core_ids=list(range(8)),
                               trace=_return_perf)
    out = np.empty((BS, N, DIM), dtype=np.float32)
    for b in range(BS):
        partial = res.results[2 * b]["out"] + res.results[2 * b + 1]["out"]
        out[b] = partial.T + proj_b[None, :]
    if _return_perf:
        return out, res
    return out
